# revision 9
# baseline (speedup 1.0000x reference)
"""GCN (2-layer, PyG GCNConv semantics) on 8 Trainium2 NeuronCores.

Strategy (dst-sharded graph parallel):
  - Nodes sharded 6250/core by destination range. Small weights replicated.
  - norm folding: out = dinv ⊙ (Σ_e w_e · (dinv ⊙ h)[src_e]) + b, so the only
    per-edge scalar is the raw edge weight w_e (folded into the scatter matrix).
  - Phase 1 (replicated): h1' = dinv ⊙ (x @ W1) for all nodes, bf16 rows in HBM.
  - Conv scatter-gather per core:  dma_gather 256B rows of h'[src] (DMA engines)
    -> PE matmul against a weighted one-hot S tile (host-packed weighted one-hot,
    streamed from HBM) accumulating per-64-dst-window PSUM.
  - Conv1 flush: relu(psum+b1) * dinv -> @W2 -> h2' shard; AllGather h2' ->
    conv2 (same structure, transposed matmul orientation) -> +b2 -> log_softmax.
Host does layout only: edge sorting/padding, int16 index tables, degree bincount.
"""

import numpy as np
import ml_dtypes
from contextlib import ExitStack

import concourse.bass as bass
import concourse.bacc as bacc
import concourse.tile as tile
from concourse import mybir
from concourse.bass_utils import run_bass_kernel_spmd

# ---------------------------------------------------------------- constants
N, E = 50000, 800000
NFEAT, NHID, NCLASS = 256, 128, 47
NCORES = 8
PER = N // NCORES            # 6250 nodes per core
WIN = 64                     # dst window (psum free dim)
NWIN = (PER + WIN - 1) // WIN  # 98
SHARD = NWIN * WIN           # 6272 rows per h2 shard (padded)
H2ROWS = NCORES * SHARD      # 50176
HALF1 = N // 2               # 25000  (conv1 gather src-half boundary)
HALF2 = H2ROWS // 2          # 25088  (conv2 row-half boundary; same src split)
NP1 = ((N + 127) // 128) * 128  # 50048 h' rows padded
CW = 7                       # windows per gather chunk
NMAX_IDX = 8192              # max idxs per dma_gather call
f32 = mybir.dt.float32
bf16 = mybir.dt.bfloat16
i16 = mybir.dt.int16
bfnp = ml_dtypes.bfloat16


# ---------------------------------------------- walrus sync-wait limit workaround
def _split_sync_waits(nc, maxw=1):
    """This walrus build has tight per-struct sync-wait slot limits; move
    overflow waits onto preceding same-engine NoOps."""
    cnt = 0
    for f in nc.m.functions:
        for b in f.blocks:
            newl = []
            changed = False
            for inst in b.instructions:
                si = inst.sync_info
                waits = list(si.on_wait) if si is not None else []
                if len(waits) > maxw:
                    changed = True
                    keep = waits[-maxw:]
                    over = waits[:-maxw]
                    for i in range(0, len(over), maxw):
                        cnt += 1
                        nop = mybir.InstNoOp(
                            name=f"wsplit_{cnt}_{inst.name}",
                            bass_nofuse=True,
                            engine=inst.engine,
                            sync_info=mybir.SyncInfo(
                                on_wait=over[i:i + maxw], on_update=[]),
                        )
                        newl.append(nop)
                    inst.sync_info = mybir.SyncInfo(
                        on_wait=keep,
                        on_update=list(si.on_update) if si is not None else [])
                newl.append(inst)
            if changed:
                b.instructions = newl
    return cnt


# ------------------------------------------------------------- host preprocessing
def _wrap_idx(idx_tiles):
    """[nt,128] int -> wrapped int16 [128, nt*8]: IDX[16k+q, t*8+r] = idx[t, r*16+q]."""
    nt = idx_tiles.shape[0]
    m = np.transpose(idx_tiles.reshape(nt, 8, 16), (2, 0, 1)).reshape(16, nt * 8)
    return np.tile(m, (8, 1)).astype(np.int16).copy()


def _preprocess(x, edge_index, edge_weight):
    src = np.concatenate([edge_index[0], np.arange(N, dtype=np.int64)])
    dst = np.concatenate([edge_index[1], np.arange(N, dtype=np.int64)])
    w = np.concatenate([edge_weight.astype(np.float32), np.ones(N, np.float32)])

    deg = (np.bincount(dst, weights=w.astype(np.float64), minlength=N)
           .astype(np.float32))  # includes self-loop weight 1

    owner = dst // PER
    loc = dst - owner * PER
    win = loc // WIN
    dstl = loc % WIN
    half = (src >= HALF1).astype(np.int64)

    # global sort: (owner, win, half, src)
    key = ((owner * NWIN + win) * 2 + half) * N + src
    order = np.argsort(key, kind="stable")
    src, w, owner, win, dstl, half = (a[order] for a in (src, w, owner, win, dstl, half))

    seg_id = (owner * NWIN + win) * 2 + half  # 0 .. NCORES*NWIN*2
    counts = np.bincount(seg_id, minlength=NCORES * NWIN * 2).reshape(NCORES, NWIN, 2)
    tiles_seg = -(-counts // 128)                       # ceil
    T = tiles_seg.max(axis=0)                          # [NWIN, 2] shared tile counts
    T = np.maximum(T, counts.max(axis=0) > 0)          # (already implied)
    # ensure each window has >= 1 tile total (self loops guarantee counts>0)
    assert (T.sum(axis=1) >= 1).all()

    ntiles = int(T.sum())
    seg_starts_global = np.concatenate([[0], np.cumsum(counts.reshape(-1))])

    # chunk structure (shared across cores)
    chunks = [(c, min(c + CW, NWIN)) for c in range(0, NWIN, CW)]
    # tile page layout per chunk: lo tiles (window-major), then hi tiles
    page_of = np.zeros((NWIN, 2), np.int64)   # first page (global) of each segment
    chunk_meta = []  # (w0, w1, lo_tiles, hi_tiles, page_base)
    pg = 0
    for (w0, w1) in chunks:
        lo = int(T[w0:w1, 0].sum())
        hi = int(T[w0:w1, 1].sum())
        base = pg
        for wi in range(w0, w1):
            page_of[wi, 0] = pg
            pg += int(T[wi, 0])
        for wi in range(w0, w1):
            page_of[wi, 1] = pg
            pg += int(T[wi, 1])
        chunk_meta.append((w0, w1, lo, hi, base))
    assert pg == ntiles

    # per-core padded arrays in tile order
    g1 = np.zeros((NCORES, ntiles, 128), np.int64)
    g2 = np.zeros((NCORES, ntiles, 128), np.int64)
    sdl = np.full((NCORES, ntiles, 128), -1, np.int64)
    sw = np.zeros((NCORES, ntiles, 128), np.float32)

    for c in range(NCORES):
        for wi in range(NWIN):
            for h in range(2):
                seg = c * NWIN * 2 + wi * 2 + h
                s0, s1 = seg_starts_global[seg], seg_starts_global[seg + 1]
                cnt = s1 - s0
                p0 = page_of[wi, h]
                if cnt == 0:
                    continue
                ssrc = src[s0:s1]
                flat = np.arange(cnt)
                tt = p0 + flat // 128
                pp = flat % 128
                g1[c, tt, pp] = ssrc - h * HALF1
                grow = (ssrc // PER) * SHARD + (ssrc % PER)
                g2[c, tt, pp] = grow - h * HALF2
                sdl[c, tt, pp] = dstl[s0:s1]
                sw[c, tt, pp] = w[s0:s1]

    assert g1.max() < 32768 and g2.max() < 32768

    pre = dict(
        deg=deg, T=T, ntiles=ntiles, chunk_meta=chunk_meta, page_of=page_of,
    )

    # per-core input tensors
    deg_all = np.ones(NP1, np.float32)
    deg_all[:N] = deg

    per_core = []
    for c in range(NCORES):
        deg_own = np.ones(SHARD, np.float32)
        deg_own[:PER] = deg[c * PER:(c + 1) * PER]
        # packed weighted one-hot S tiles: [128, ntiles*WIN] bf16
        spack = np.zeros((128, ntiles * WIN), bfnp)
        tt, pp = np.nonzero(sdl[c] >= 0)
        spack[pp, tt * WIN + sdl[c, tt, pp]] = sw[c, tt, pp].astype(bfnp)
        per_core.append(dict(
            idx1=_wrap_idx(g1[c]),
            idx2=_wrap_idx(g2[c]),
            spack=spack,
            deg_own=deg_own,
        ))
    shared = dict(deg_all=deg_all)
    return pre, shared, per_core


# ------------------------------------------------------------------ program build
def _build_program(pre):
    T = pre["T"]; chunk_meta = pre["chunk_meta"]; page_of = pre["page_of"]
    ntiles = pre["ntiles"]

    nc = bacc.Bacc("TRN2", target_bir_lowering=False, debug=False,
                   num_devices=NCORES)

    xt_in = nc.dram_tensor("xt", [NFEAT, NP1], bf16, kind="ExternalInput")
    w1_in = nc.dram_tensor("w1", [NFEAT, NHID], bf16, kind="ExternalInput")
    w2_in = nc.dram_tensor("w2", [NHID, 64], bf16, kind="ExternalInput")
    b1_in = nc.dram_tensor("b1", [NHID], f32, kind="ExternalInput")
    b2bc_in = nc.dram_tensor("b2bc", [WIN, 128], f32, kind="ExternalInput")
    dega_in = nc.dram_tensor("deg_all", [NP1], f32, kind="ExternalInput")
    dego_in = nc.dram_tensor("deg_own", [SHARD], f32, kind="ExternalInput")
    idx1_in = nc.dram_tensor("idx1", [128, ntiles * 8], i16, kind="ExternalInput")
    idx2_in = nc.dram_tensor("idx2", [128, ntiles * 8], i16, kind="ExternalInput")
    spack_in = nc.dram_tensor("spack", [128, ntiles * WIN], bf16, kind="ExternalInput")

    out_d = nc.dram_tensor("out", [SHARD, NCLASS], f32, kind="ExternalOutput")

    hp1 = nc.dram_tensor("hp1", [NP1, NHID], bf16)
    dinv_dr = nc.dram_tensor("dinv_dr", [SHARD], f32)
    h2shard = nc.dram_tensor("h2shard", [SHARD, 128], bf16)
    h2full = nc.dram_tensor("h2full", [H2ROWS, 128], bf16, addr_space="Shared")

    with tile.TileContext(nc) as tc, ExitStack() as ctx:
        cpool = ctx.enter_context(tc.tile_pool(name="consts", bufs=1))
        xpool = ctx.enter_context(tc.tile_pool(name="xload", bufs=3))
        hpool = ctx.enter_context(tc.tile_pool(name="hstage", bufs=3))
        p1ps = ctx.enter_context(tc.tile_pool(name="p1ps", bufs=2, space="PSUM"))
        gpool = ctx.enter_context(tc.tile_pool(name="gmsg", bufs=2))
        spool = ctx.enter_context(tc.tile_pool(name="sbuild", bufs=3))
        fpool = ctx.enter_context(tc.tile_pool(name="flush", bufs=3))
        wps = ctx.enter_context(tc.tile_pool(name="wps", bufs=2, space="PSUM"))

        # ---- constants ----
        w1_sb = cpool.tile([128, 2, NHID], bf16)
        nc.sync.dma_start(out=w1_sb[:], in_=w1_in.ap().rearrange("(c p) h -> p c h", p=128))
        w2_sb = cpool.tile([128, 64], bf16)
        nc.sync.dma_start(out=w2_sb[:], in_=w2_in.ap())
        b1_sb = cpool.tile([128, 1], f32)
        nc.sync.dma_start(out=b1_sb[:], in_=b1_in.ap().unsqueeze(1))
        b2bc_sb = cpool.tile([WIN, 128], f32)
        nc.sync.dma_start(out=b2bc_sb[:], in_=b2bc_in.ap())
        idx1_sb = cpool.tile([128, ntiles * 8], i16)
        nc.sync.dma_start(out=idx1_sb[:], in_=idx1_in.ap())
        idx2_sb = cpool.tile([128, ntiles * 8], i16)
        nc.sync.dma_start(out=idx2_sb[:], in_=idx2_in.ap())

        # dinv (all nodes, node-on-partition): [128, NP1/128]
        njt = NP1 // 128
        deg_sb = cpool.tile([128, njt], f32)
        nc.sync.dma_start(out=deg_sb[:], in_=dega_in.ap().rearrange("(j p) -> p j", p=128))
        rec = cpool.tile([128, njt], f32)
        nc.vector.reciprocal(rec[:], deg_sb[:])
        dinv_nw = cpool.tile([128, njt], f32)
        nc.scalar.sqrt(dinv_nw[:], rec[:])

        # dinv own: [64, NWIN] then scratch->broadcast [128, SHARD]
        dego_sb = cpool.tile([WIN, NWIN], f32)
        nc.sync.dma_start(out=dego_sb[:], in_=dego_in.ap().rearrange("(j p) -> p j", p=WIN))
        reco = cpool.tile([WIN, NWIN], f32)
        nc.vector.reciprocal(reco[:], dego_sb[:])
        dinv_p64 = cpool.tile([WIN, NWIN], f32)
        nc.scalar.sqrt(dinv_p64[:], reco[:])
        nc.sync.dma_start(out=dinv_dr.ap().rearrange("(j p) -> p j", p=WIN), in_=dinv_p64[:])
        dinv_bc = cpool.tile([128, SHARD], f32)
        nc.sync.dma_start(out=dinv_bc[:],
                          in_=dinv_dr.ap().unsqueeze(0).broadcast_to([128, SHARD]))

        # ---- phase 1: h1' = dinv * (x @ W1), all nodes ----
        GN = 512
        for g0 in range(0, NP1, GN):
            gn = min(GN, NP1 - g0)
            nt = gn // 128
            xt_t = xpool.tile([128, 2, GN], bf16, tag="xt")
            nc.sync.dma_start(
                out=xt_t[:, :, :gn],
                in_=xt_in.ap()[:, g0:g0 + gn].rearrange("(c p) n -> p c n", p=128))
            hp_t = hpool.tile([128, GN // 128, NHID], bf16, tag="hp")
            for t in range(nt):
                ps = p1ps.tile([128, NHID], f32, tag="p1")
                for cch in range(2):
                    nc.tensor.matmul(ps[:],
                                     lhsT=xt_t[:, cch, t * 128:(t + 1) * 128],
                                     rhs=w1_sb[:, cch, :],
                                     start=(cch == 0), stop=(cch == 1))
                nc.vector.tensor_scalar(hp_t[:, t, :], ps[:],
                                        dinv_nw[:, (g0 // 128 + t):(g0 // 128 + t) + 1],
                                        None, op0=mybir.AluOpType.mult)
            nc.sync.dma_start(
                out=hp1.ap()[g0:g0 + gn, :].rearrange("(t p) h -> p t h", p=128),
                in_=hp_t[:, :nt, :])

        # ---- conv passes ----
        reg_cache = {}

        def reg_of(v):
            if v not in reg_cache:
                reg_cache[v] = nc.gpsimd.to_reg(v)
            return reg_cache[v]

        def conv(pass_idx, idx_sb, src0_ap, src1_ap):
            for (w0, w1, nlo, nhi, base) in chunk_meta:
                nall = nlo + nhi
                if nall == 0:
                    continue
                msgs = gpool.tile([128, nall, 128], bf16, tag=f"msgs{pass_idx}")
                tmax_call = NMAX_IDX // 128
                for h, hn, hoff, src_ap in ((0, nlo, 0, src0_ap),
                                            (1, nhi, nlo, src1_ap)):
                    for t0 in range(0, hn, tmax_call):
                        tn = min(tmax_call, hn - t0)
                        p0 = base + hoff + t0
                        nc.gpsimd.dma_gather(
                            out_ap=msgs[:, hoff + t0:hoff + t0 + tn, :],
                            in_ap=src_ap,
                            idxs_ap=idx_sb[:, p0 * 8:(p0 + tn) * 8],
                            num_idxs=tn * 128, num_idxs_reg=reg_of(tn * 128),
                            elem_size=128, single_packet=False)
                stk = spool.tile([128, nall * WIN], bf16, tag=f"stk{pass_idx}")
                nc.sync.dma_start(
                    out=stk[:],
                    in_=spack_in.ap()[:, base * WIN:(base + nall) * WIN])
                for wi in range(w0, w1):
                    tlo, thi = int(T[wi, 0]), int(T[wi, 1])
                    twin = tlo + thi
                    if twin == 0:
                        continue
                    # tile pages of this window within the chunk (lo block, hi block)
                    pages = ([(page_of[wi, 0] - base) + k for k in range(tlo)]
                             + [(page_of[wi, 1] - base) + k for k in range(thi)])
                    if pass_idx == 1:
                        ps1 = wps.tile([128, WIN], f32, tag="ps1")
                        for k, pg in enumerate(pages):
                            nc.tensor.matmul(ps1[:], lhsT=msgs[:, pg, :],
                                             rhs=stk[:, pg * WIN:(pg + 1) * WIN],
                                             start=(k == 0), stop=(k == twin - 1))
                        # flush: o1 = psum*dinv + b1 ; rs = dinv * relu(o1)
                        u_sb = fpool.tile([128, WIN], f32, tag="u1")
                        nc.vector.tensor_mul(u_sb[:], ps1[:],
                                             dinv_bc[:, wi * WIN:(wi + 1) * WIN])
                        r_sb = fpool.tile([128, WIN], f32, tag="r1")
                        nc.scalar.activation(r_sb[:], u_sb[:],
                                             mybir.ActivationFunctionType.Relu,
                                             bias=b1_sb[:, 0:1], scale=1.0)
                        rs_sb = fpool.tile([128, WIN], bf16, tag="rs1")
                        nc.vector.tensor_mul(rs_sb[:], r_sb[:],
                                             dinv_bc[:, wi * WIN:(wi + 1) * WIN])
                        ps2 = wps.tile([WIN, 64], f32, tag="ps2")
                        nc.tensor.matmul(ps2[:], lhsT=rs_sb[:], rhs=w2_sb[:],
                                         start=True, stop=True)
                        h2t = fpool.tile([WIN, 128], bf16, tag="h2t")
                        nc.vector.memset(h2t[:, 64:], 0.0)
                        nc.vector.tensor_copy(h2t[:, :64], ps2[:])
                        nc.sync.dma_start(
                            out=h2shard.ap()[wi * WIN:(wi + 1) * WIN, :], in_=h2t[:])
                    else:
                        ps3 = wps.tile([WIN, 128], f32, tag="ps3")
                        for k, pg in enumerate(pages):
                            nc.tensor.matmul(ps3[:], lhsT=stk[:, pg * WIN:(pg + 1) * WIN],
                                             rhs=msgs[:, pg, :],
                                             start=(k == 0), stop=(k == twin - 1))
                        u_sb = fpool.tile([WIN, 128], f32, tag="u2")
                        nc.vector.tensor_scalar(u_sb[:], ps3[:],
                                                dinv_p64[:, wi:wi + 1], None,
                                                op0=mybir.AluOpType.mult)
                        o_sb = fpool.tile([WIN, 128], f32, tag="o2")
                        nc.vector.tensor_add(o_sb[:], u_sb[:], b2bc_sb[:])
                        nm = fpool.tile([WIN, 1], f32, tag="nm")
                        nc.vector.tensor_reduce(nm[:], o_sb[:, :NCLASS],
                                                axis=mybir.AxisListType.X,
                                                op=mybir.AluOpType.max, negate=True)
                        ex = fpool.tile([WIN, NCLASS], f32, tag="ex")
                        ssum = fpool.tile([WIN, 1], f32, tag="ssum")
                        nc.scalar.activation(ex[:], o_sb[:, :NCLASS],
                                             mybir.ActivationFunctionType.Exp,
                                             bias=nm[:], scale=1.0, accum_out=ssum[:])
                        lns = fpool.tile([WIN, 1], f32, tag="lns")
                        nc.scalar.activation(lns[:], ssum[:],
                                             mybir.ActivationFunctionType.Ln)
                        res = fpool.tile([WIN, NCLASS], f32, tag="res")
                        nc.vector.tensor_scalar(res[:], o_sb[:, :NCLASS], nm[:], lns[:],
                                                op0=mybir.AluOpType.add,
                                                op1=mybir.AluOpType.subtract)
                        nc.sync.dma_start(
                            out=out_d.ap()[wi * WIN:(wi + 1) * WIN, :], in_=res[:])

        conv(1, idx1_sb, hp1.ap(), hp1.ap()[HALF1:, :])

        nc.gpsimd.collective_compute(
            "AllGather", mybir.AluOpType.bypass,
            replica_groups=[list(range(NCORES))],
            ins=[h2shard.ap()], outs=[h2full.ap()])

        conv(2, idx2_sb, h2full.ap(), h2full.ap()[HALF2:, :])

    return nc


# ------------------------------------------------------------------ entry point
_CACHE = {}


def kernel(x, edge_index, edge_weight, W1, b1, W2, b2, _profile=False):
    pre, shared, per_core = _preprocess(
        np.asarray(x), np.asarray(edge_index).astype(np.int64),
        np.asarray(edge_weight))

    nc = _build_program(pre)
    nc.finalize()
    _split_sync_waits(nc)

    xt = np.zeros((NFEAT, NP1), bfnp)
    xt[:, :N] = np.asarray(x).astype(bfnp).T
    w1b = np.asarray(W1).astype(bfnp)
    w2b = np.zeros((NHID, 64), bfnp)
    w2b[:, :NCLASS] = np.asarray(W2).astype(bfnp)
    b2bc = np.zeros((WIN, 128), np.float32)
    b2bc[:, :NCLASS] = np.asarray(b2)[None, :]

    in_maps = []
    for c in range(NCORES):
        m = dict(
            xt=xt, w1=w1b, w2=w2b, b1=np.asarray(b1).astype(np.float32),
            b2bc=b2bc, deg_all=shared["deg_all"],
            idx1=per_core[c]["idx1"], idx2=per_core[c]["idx2"],
            spack=per_core[c]["spack"], deg_own=per_core[c]["deg_own"],
        )
        in_maps.append(m)

    r = run_bass_kernel_spmd(nc, in_maps, list(range(NCORES)), trace=_profile)
    _CACHE["last_result"] = r

    out = np.concatenate([r.results[c]["out"][:PER] for c in range(NCORES)], axis=0)
    return out.astype(np.float32)
